# revision 13
# baseline (speedup 1.0000x reference)
"""Trainium2 Bass kernel for nn_AttentionLayer (B=64, F=1024, K=1024).

Reference computation (per batch b):
    scores[k, g] = sum_f input[b, f, k] * weight[f, g] + bias[g]
    alpha        = softmax(scores, axis=g)
    out[b, f, k] = input[b, f, k] * alpha[k, f]

Strategy: data-parallel over batch across 8 NeuronCores (8 batches/core).
Per batch, everything is computed in the transposed [g, k] layout so that no
transposes are ever needed:
    scoresT[g, k] = sum_f W[f, g] * X[f, k]      (lhsT = W chunk, rhs = X chunk)
    E[g, k]  = exp(scoresT + bias[g])            (ScalarE, bias is per-partition)
    S[., k]  = sum_g E[g, k]                     (matmul with ones[128,128]
                                                  stationary -> sum replicated
                                                  across partitions = free bcast)
    D = 1/S  (reciprocal_approx_fast)
    out[f, k] = X[f, k] * E[f, k] * D[k]         (VectorE, g === f axis)

Matmuls run with float32r operands (1 cyc/row vs 4 for fp32).
"""

import sys
from contextlib import ExitStack

import numpy as np

for _p in ("/opt/trn_rl_repo", "/root/.axon_site/_ro/trn_rl_repo"):
    if _p not in sys.path:
        sys.path.append(_p)

import concourse.bacc as bacc
import concourse.bass as bass
import concourse.mybir as mybir
import concourse.tile as tile
from concourse.bass_utils import run_bass_kernel_spmd

N_CORES = 8
B, F, K = 64, 1024, 1024
BPC = B // N_CORES            # batches per core
P = 128                       # SBUF partitions
NF = F // P                   # f (contraction) chunks
NG = F // P                   # g (feature/output-partition) chunks
KC = 512                      # moving free-dim chunk (fp32 max = 512)
NK = K // KC

FP32 = mybir.dt.float32
F32R = mybir.dt.float32r

EXP = mybir.ActivationFunctionType.Exp


def _build(mm_f32r: bool = True, bpc: int = BPC):
    nc = bacc.Bacc("TRN2", target_bir_lowering=False, debug=False)

    mmdt = F32R if mm_f32r else FP32
    x_d = nc.dram_tensor("x", [bpc, F, K], mmdt, kind="ExternalInput").ap()
    w_d = nc.dram_tensor("w", [F, F], mmdt, kind="ExternalInput").ap()
    b_d = nc.dram_tensor("b", [1, F], FP32, kind="ExternalInput").ap()
    ones_d = nc.dram_tensor("ones", [P, P], mmdt, kind="ExternalInput").ap()
    o_d = nc.dram_tensor("out", [bpc, F, K], FP32, kind="ExternalOutput").ap()

    def as_f32(ap):
        return ap.bitcast(FP32) if mm_f32r else ap

    with tile.TileContext(nc) as tc, ExitStack() as ctx:
        w_pool = ctx.enter_context(tc.tile_pool(name="w", bufs=1))
        c_pool = ctx.enter_context(tc.tile_pool(name="const", bufs=1))
        x_pool = ctx.enter_context(tc.tile_pool(name="x", bufs=2))
        e_pool = ctx.enter_context(tc.tile_pool(name="e", bufs=16))
        d_pool = ctx.enter_context(tc.tile_pool(name="d", bufs=2))
        o_pool = ctx.enter_context(tc.tile_pool(name="o", bufs=2))
        sc_psum = ctx.enter_context(tc.tile_pool(name="sc", bufs=3, space="PSUM"))
        s_psum = ctx.enter_context(tc.tile_pool(name="s", bufs=1, space="PSUM"))

        # ---- constants / weights (loaded once) ----
        # w_sb[p, fc*F + g] = W[fc*128 + p, g]
        w_sb = w_pool.tile([P, NF * F], mmdt)
        for fc in range(NF):
            nc.sync.dma_start(
                out=w_sb[:, fc * F : (fc + 1) * F],
                in_=w_d[fc * P : (fc + 1) * P, :],
            )
        # bias_sb[p, gc] = bias[gc*128 + p]
        bias_sb = c_pool.tile([P, NG], FP32)
        nc.sync.dma_start(
            out=bias_sb[:], in_=b_d.rearrange("o (c p) -> (o p) c", p=P)
        )
        ones_sb = c_pool.tile([P, P], mmdt)
        nc.sync.dma_start(out=ones_sb[:], in_=ones_d)

        def w_tile(fc, gc):
            off = fc * F + gc * P
            return w_sb[:, off : off + P]

        def stage_a(b):
            """DMA in X[b], main matmuls, exp -> (x_t, e_tiles)."""
            x_t = x_pool.tile([P, NF * K], mmdt, tag="x")
            for fc in range(NF):
                nc.sync.dma_start(
                    out=x_t[:, fc * K : (fc + 1) * K],
                    in_=x_d[b, fc * P : (fc + 1) * P, :],
                )
            e_tiles = []
            for gc in range(NG):
                sc = sc_psum.tile([P, K], FP32, tag="sc")
                for kc in range(NK):
                    for fc in range(NF):
                        nc.tensor.matmul(
                            sc[:, kc * KC : (kc + 1) * KC],
                            lhsT=w_tile(fc, gc),
                            rhs=x_t[:, fc * K + kc * KC : fc * K + kc * KC + KC],
                            start=(fc == 0),
                            stop=(fc == NF - 1),
                        )
                e_t = e_pool.tile([P, K], mmdt, tag="e")
                nc.scalar.activation(
                    e_t[:], sc[:], EXP, bias=bias_sb[:, gc : gc + 1], scale=1.0
                )
                e_tiles.append(e_t)
            return x_t, e_tiles

        def stage_b(b, x_t, e_tiles):
            """Partition-sum, reciprocal, final muls, DMA out."""
            s_t = s_psum.tile([P, K], FP32, tag="s")
            for kc in range(NK):
                for gc in range(NG):
                    nc.tensor.matmul(
                        s_t[:, kc * KC : (kc + 1) * KC],
                        lhsT=ones_sb[:],
                        rhs=e_tiles[gc][:, kc * KC : (kc + 1) * KC],
                        start=(gc == 0),
                        stop=(gc == NG - 1),
                    )
            d_t = d_pool.tile([P, K], FP32, tag="d")
            nc.vector.reciprocal_approx_fast(d_t[:], s_t[:])
            for fc in range(NF):
                o_t = o_pool.tile([P, K], FP32, tag="o")
                nc.vector.tensor_mul(
                    o_t[:],
                    as_f32(x_t[:, fc * K : (fc + 1) * K]),
                    as_f32(e_tiles[fc][:]),
                )
                nc.vector.tensor_mul(o_t[:], o_t[:], d_t[:])
                nc.sync.dma_start(out=o_d[b, fc * P : (fc + 1) * P, :], in_=o_t[:])

        # software pipeline: PE stream is [mainMM b][sumMM b-1][mainMM b+1]...
        prev = None
        for b in range(bpc):
            cur = stage_a(b)
            if prev is not None:
                stage_b(b - 1, *prev)
            prev = cur
        stage_b(bpc - 1, *prev)

    nc.compile()
    return nc


_NC = None


def _get_nc():
    global _NC
    if _NC is None:
        _NC = _build()
    return _NC


def kernel(**inputs) -> np.ndarray:
    x = np.ascontiguousarray(np.asarray(inputs["input"], dtype=np.float32))
    w = np.ascontiguousarray(np.asarray(inputs["weight"], dtype=np.float32))
    b = np.ascontiguousarray(np.asarray(inputs["bias"], dtype=np.float32))

    nc = _get_nc()
    ones = np.ones((P, P), dtype=np.float32)
    in_maps = [
        {"x": x[c * BPC : (c + 1) * BPC], "w": w, "b": b, "ones": ones}
        for c in range(N_CORES)
    ]
    res = run_bass_kernel_spmd(nc, in_maps, list(range(N_CORES)))
    return np.concatenate([res.results[c]["out"] for c in range(N_CORES)], axis=0)


# revision 16
# speedup vs baseline: 8.8421x; 8.8421x over previous
"""Trainium2 Bass kernel for nn_AttentionLayer (B=64, F=1024, K=1024).

Reference computation (per batch b):
    scores[k, g] = sum_f input[b, f, k] * weight[f, g] + bias[g]
    alpha        = softmax(scores, axis=g)
    out[b, f, k] = input[b, f, k] * alpha[k, f]

Strategy: data-parallel over batch across 8 NeuronCores (8 batches/core).
Per batch, everything is computed in the transposed [g, k] layout so that no
transposes are ever needed:
    scoresT[g, k] = sum_f W[f, g] * X[f, k]      (lhsT = W chunk, rhs = X chunk)
    E[g, k]  = exp(scoresT + bias[g])            (ScalarE, bias is per-partition)
    S[., k]  = sum_g E[g, k]                     (matmul with ones[128,128]
                                                  stationary -> sum replicated
                                                  across partitions = free bcast)
    D = 1/S  (reciprocal_approx_fast)
    out[f, k] = X[f, k] * E[f, k] * D[k]         (VectorE, g === f axis)

Matmuls run with float32r operands (1 cyc/row vs 4 for fp32).
"""

import sys
from contextlib import ExitStack

import numpy as np

for _p in ("/opt/trn_rl_repo", "/root/.axon_site/_ro/trn_rl_repo"):
    if _p not in sys.path:
        sys.path.append(_p)

import concourse.bacc as bacc
import concourse.bass as bass
import concourse.mybir as mybir
import concourse.tile as tile
from concourse.bass_utils import run_bass_kernel_spmd

N_CORES = 8
B, F, K = 64, 1024, 1024
BPC = B // N_CORES            # batches per core
P = 128                       # SBUF partitions
NF = F // P                   # f (contraction) chunks
NG = F // P                   # g (feature/output-partition) chunks
KC = 512                      # moving free-dim chunk (fp32 max = 512)
NK = K // KC

FP32 = mybir.dt.float32
F32R = mybir.dt.float32r

EXP = mybir.ActivationFunctionType.Exp


def _build(mm_f32r: bool = True, bpc: int = BPC, reps: int = 1):
    nc = bacc.Bacc("TRN2", target_bir_lowering=False, debug=False)

    mmdt = F32R if mm_f32r else FP32
    x_d = nc.dram_tensor("x", [bpc, F, K], mmdt, kind="ExternalInput").ap()
    w_d = nc.dram_tensor("w", [F, F], mmdt, kind="ExternalInput").ap()
    b_d = nc.dram_tensor("b", [1, F], FP32, kind="ExternalInput").ap()
    ones_d = nc.dram_tensor("ones", [P, P], mmdt, kind="ExternalInput").ap()
    o_d = nc.dram_tensor("out", [bpc, F, K], FP32, kind="ExternalOutput").ap()

    def as_f32(ap):
        return ap.bitcast(FP32) if mm_f32r else ap

    with tile.TileContext(nc) as tc, ExitStack() as ctx:
        w_pool = ctx.enter_context(tc.tile_pool(name="w", bufs=1))
        c_pool = ctx.enter_context(tc.tile_pool(name="const", bufs=1))
        x_pool = ctx.enter_context(tc.tile_pool(name="x", bufs=2))
        e_pool = ctx.enter_context(tc.tile_pool(name="e", bufs=16))
        d_pool = ctx.enter_context(tc.tile_pool(name="d", bufs=2))
        o_pool = ctx.enter_context(tc.tile_pool(name="o", bufs=2))
        sc_psum = ctx.enter_context(tc.tile_pool(name="sc", bufs=3, space="PSUM"))
        s_psum = ctx.enter_context(tc.tile_pool(name="s", bufs=1, space="PSUM"))

        # ---- constants / weights (loaded once) ----
        # w_sb[p, fc*F + g] = W[fc*128 + p, g]
        w_sb = w_pool.tile([P, NF * F], mmdt)
        for fc in range(NF):
            nc.sync.dma_start(
                out=w_sb[:, fc * F : (fc + 1) * F],
                in_=w_d[fc * P : (fc + 1) * P, :],
            )
        # bias_sb[p, gc] = bias[gc*128 + p]
        bias_sb = c_pool.tile([P, NG], FP32)
        nc.sync.dma_start(
            out=bias_sb[:], in_=b_d.rearrange("o (c p) -> (o p) c", p=P)
        )
        ones_sb = c_pool.tile([P, P], mmdt)
        nc.sync.dma_start(out=ones_sb[:], in_=ones_d)

        def w_tile(fc, gc):
            off = fc * F + gc * P
            return w_sb[:, off : off + P]

        def stage_a(b):
            """DMA in X[b], main matmuls, exp -> (x_t, e_tiles)."""
            x_t = x_pool.tile([P, NF * K], mmdt, tag="x")
            for fc in range(NF):
                nc.sync.dma_start(
                    out=x_t[:, fc * K : (fc + 1) * K],
                    in_=x_d[b, fc * P : (fc + 1) * P, :],
                )
            e_tiles = []
            for gc in range(NG):
                sc = sc_psum.tile([P, K], FP32, tag="sc")
                for kc in range(NK):
                    for fc in range(NF):
                        nc.tensor.matmul(
                            sc[:, kc * KC : (kc + 1) * KC],
                            lhsT=w_tile(fc, gc),
                            rhs=x_t[:, fc * K + kc * KC : fc * K + kc * KC + KC],
                            start=(fc == 0),
                            stop=(fc == NF - 1),
                        )
                e_t = e_pool.tile([P, K], mmdt, tag="e")
                nc.scalar.activation(
                    e_t[:], sc[:], EXP, bias=bias_sb[:, gc : gc + 1], scale=1.0
                )
                e_tiles.append(e_t)
            return x_t, e_tiles

        def stage_b(b, x_t, e_tiles):
            """Partition-sum, reciprocal, final muls, DMA out."""
            s_t = s_psum.tile([P, K], FP32, tag="s")
            for kc in range(NK):
                for gc in range(NG):
                    nc.tensor.matmul(
                        s_t[:, kc * KC : (kc + 1) * KC],
                        lhsT=ones_sb[:],
                        rhs=e_tiles[gc][:, kc * KC : (kc + 1) * KC],
                        start=(gc == 0),
                        stop=(gc == NG - 1),
                    )
            d_t = d_pool.tile([P, K], FP32, tag="d")
            nc.vector.reciprocal_approx_fast(d_t[:], s_t[:])
            for fc in range(NF):
                o_t = o_pool.tile([P, K], FP32, tag="o")
                nc.vector.tensor_mul(
                    o_t[:],
                    as_f32(x_t[:, fc * K : (fc + 1) * K]),
                    as_f32(e_tiles[fc][:]),
                )
                nc.vector.tensor_mul(o_t[:], o_t[:], d_t[:])
                nc.sync.dma_start(out=o_d[b, fc * P : (fc + 1) * P, :], in_=o_t[:])

        # software pipeline: PE stream is [mainMM b][sumMM b-1][mainMM b+1]...
        prev = None
        for _ in range(reps):
            for b in range(bpc):
                cur = (b, *stage_a(b))
                if prev is not None:
                    stage_b(*prev)
                prev = cur
        stage_b(*prev)

    nc.compile()
    return nc


_NC = None


def _get_nc():
    global _NC
    if _NC is None:
        _NC = _build()
    return _NC


def kernel(**inputs) -> np.ndarray:
    x = np.ascontiguousarray(np.asarray(inputs["input"], dtype=np.float32))
    w = np.ascontiguousarray(np.asarray(inputs["weight"], dtype=np.float32))
    b = np.ascontiguousarray(np.asarray(inputs["bias"], dtype=np.float32))

    nc = _get_nc()
    ones = np.ones((P, P), dtype=np.float32)
    in_maps = [
        {"x": x[c * BPC : (c + 1) * BPC], "w": w, "b": b, "ones": ones}
        for c in range(N_CORES)
    ]
    res = run_bass_kernel_spmd(nc, in_maps, list(range(N_CORES)))
    return np.concatenate([res.results[c]["out"] for c in range(N_CORES)], axis=0)


# revision 24
# speedup vs baseline: 9.9106x; 1.1208x over previous
"""Trainium2 Bass kernel for nn_AttentionLayer (B=64, F=1024, K=1024).

Reference computation (per batch b):
    scores[k, g] = sum_f input[b, f, k] * weight[f, g] + bias[g]
    alpha        = softmax(scores, axis=g)
    out[b, f, k] = input[b, f, k] * alpha[k, f]

Strategy: data-parallel over batch across 8 NeuronCores (8 batches/core).
Per batch, everything is computed in the transposed [g, k] layout so that no
transposes are ever needed:
    scoresT[g, k] = sum_f W[f, g] * X[f, k]      (lhsT = W chunk, rhs = X chunk)
    E[g, k]  = exp(scoresT + bias[g])            (ScalarE, bias is per-partition)
    S[., k]  = sum_g E[g, k]                     (matmul with ones[128,128]
                                                  stationary -> sum replicated
                                                  across partitions = free bcast)
    D = 1/S  (reciprocal_approx_fast)
    out[f, k] = X[f, k] * E[f, k] * D[k]         (VectorE, g === f axis)

Matmuls run with float32r operands (1 cyc/row vs 4 for fp32).
"""

import sys
from contextlib import ExitStack

import numpy as np

for _p in ("/opt/trn_rl_repo", "/root/.axon_site/_ro/trn_rl_repo"):
    if _p not in sys.path:
        sys.path.append(_p)

import concourse.bacc as bacc
import concourse.bass as bass
import concourse.mybir as mybir
import concourse.tile as tile
from concourse.bass_utils import run_bass_kernel_spmd

N_CORES = 8
B, F, K = 64, 1024, 1024
BPC = B // N_CORES            # batches per core
P = 128                       # SBUF partitions
NF = F // P                   # f (contraction) chunks
NG = F // P                   # g (feature/output-partition) chunks
KC = 512                      # moving free-dim chunk (fp32 max = 512)
NK = K // KC

FP32 = mybir.dt.float32
F32R = mybir.dt.float32r

EXP = mybir.ActivationFunctionType.Exp


def _build(mm_f32r: bool = True, bpc: int = BPC, reps: int = 1):
    nc = bacc.Bacc("TRN2", target_bir_lowering=False, debug=False)

    mmdt = F32R if mm_f32r else FP32
    x_d = nc.dram_tensor("x", [bpc, F, K], mmdt, kind="ExternalInput").ap()
    w_d = nc.dram_tensor("w", [F, F], mmdt, kind="ExternalInput").ap()
    b_d = nc.dram_tensor("b", [1, F], FP32, kind="ExternalInput").ap()
    ones_d = nc.dram_tensor("ones", [P, P], mmdt, kind="ExternalInput").ap()
    o_d = nc.dram_tensor("out", [bpc, F, K], FP32, kind="ExternalOutput").ap()

    def as_f32(ap):
        return ap.bitcast(FP32) if mm_f32r else ap

    with tile.TileContext(nc) as tc, ExitStack() as ctx:
        w_pool = ctx.enter_context(tc.tile_pool(name="w", bufs=1))
        c_pool = ctx.enter_context(tc.tile_pool(name="const", bufs=1))
        x_pool = ctx.enter_context(tc.tile_pool(name="x", bufs=16))
        e_pool = ctx.enter_context(tc.tile_pool(name="e", bufs=24))
        d_pool = ctx.enter_context(tc.tile_pool(name="d", bufs=3))
        o_pool = ctx.enter_context(tc.tile_pool(name="o", bufs=4))
        sc_psum = ctx.enter_context(tc.tile_pool(name="sc", bufs=6, space="PSUM"))
        s_psum = ctx.enter_context(tc.tile_pool(name="s", bufs=2, space="PSUM"))

        # ---- constants (loaded once; W is interleaved with batch-0 X below)
        bias_sb = c_pool.tile([P, NG], FP32)
        nc.sync.dma_start(
            out=bias_sb[:], in_=b_d.rearrange("o (c p) -> (o p) c", p=P)
        )
        ones_sb = c_pool.tile([P, P], mmdt)
        nc.sync.dma_start(out=ones_sb[:], in_=ones_d)

        # w_sb[p, fc*F + g] = W[fc*128 + p, g]
        w_sb = w_pool.tile([P, NF * F], mmdt)

        def w_tile(fc, gc):
            off = fc * F + gc * P
            return w_sb[:, off : off + P]

        def prefetch_x(b, with_w=False):
            x_tiles = []
            for fc in range(NF):
                x_t = x_pool.tile([P, K], mmdt, tag="x")
                if with_w:
                    # startup: interleave W chunks with batch-0 X, and bring
                    # the kc=0 halves in first so slab 0 starts sooner
                    nc.sync.dma_start(
                        out=w_sb[:, fc * F : (fc + 1) * F],
                        in_=w_d[fc * P : (fc + 1) * P, :],
                    )
                    nc.sync.dma_start(
                        out=x_t[:, 0:KC], in_=x_d[b, fc * P : (fc + 1) * P, 0:KC]
                    )
                else:
                    nc.sync.dma_start(
                        out=x_t[:], in_=x_d[b, fc * P : (fc + 1) * P, :]
                    )
                x_tiles.append(x_t)
            if with_w:
                for fc in range(NF):
                    nc.sync.dma_start(
                        out=x_tiles[fc][:, KC:K],
                        in_=x_d[b, fc * P : (fc + 1) * P, KC:K],
                    )
            return x_tiles

        def slab_main(b, kc, x_tiles):
            """Main matmuls + exp for one (batch, k-half) slab -> e_tiles."""
            ks = slice(kc * KC, (kc + 1) * KC)
            e_tiles = []
            for gc in range(NG):
                sc = sc_psum.tile([P, KC], FP32, tag="sc")
                for fc in range(NF):
                    nc.tensor.matmul(
                        sc[:],
                        lhsT=w_tile(fc, gc),
                        rhs=x_tiles[fc][:, ks],
                        start=(fc == 0),
                        stop=(fc == NF - 1),
                    )
                e_t = e_pool.tile([P, KC], mmdt, tag="e")
                nc.scalar.activation(
                    e_t[:], sc[:], EXP, bias=bias_sb[:, gc : gc + 1], scale=1.0
                )
                e_tiles.append(e_t)
            return e_tiles

        def slab_out(b, kc, x_tiles, e_tiles):
            """Partition-sum matmuls + reciprocal + final muls + DMA out."""
            ks = slice(kc * KC, (kc + 1) * KC)
            s_t = s_psum.tile([P, KC], FP32, tag="s")
            for gc in range(NG):
                nc.tensor.matmul(
                    s_t[:],
                    lhsT=ones_sb[:],
                    rhs=e_tiles[gc][:],
                    start=(gc == 0),
                    stop=(gc == NG - 1),
                )
            d_t = d_pool.tile([P, KC], FP32, tag="d")
            nc.vector.reciprocal_approx_fast(d_t[:], s_t[:])
            for fc in range(NF):
                o_t = o_pool.tile([P, KC], FP32, tag="o")
                nc.vector.tensor_mul(
                    o_t[:], as_f32(x_tiles[fc][:, ks]), as_f32(e_tiles[fc][:])
                )
                nc.vector.tensor_mul(o_t[:], o_t[:], d_t[:])
                nc.sync.dma_start(
                    out=o_d[b, fc * P : (fc + 1) * P, ks], in_=o_t[:]
                )

        # software pipeline over half-batch slabs: the PE stream is
        # [sums s-1][mains s][sums s][mains s+1]... so each slab's DVE chain
        # (recip + muls) overlaps the next slab's matmuls, and the kernel
        # tail after the very last main matmul is only one slab's epilogue.
        prev = None
        first = True
        for _ in range(reps):
            for b in range(bpc):
                x_tiles = prefetch_x(b, with_w=first)
                first = False
                for kc in range(NK):
                    if prev is not None:
                        slab_out(*prev)
                    e_tiles = slab_main(b, kc, x_tiles)
                    prev = (b, kc, x_tiles, e_tiles)
        slab_out(*prev)

    nc.compile()
    return nc


_NC = None


def _get_nc():
    global _NC
    if _NC is None:
        _NC = _build()
    return _NC


def kernel(**inputs) -> np.ndarray:
    x = np.ascontiguousarray(np.asarray(inputs["input"], dtype=np.float32))
    w = np.ascontiguousarray(np.asarray(inputs["weight"], dtype=np.float32))
    b = np.ascontiguousarray(np.asarray(inputs["bias"], dtype=np.float32))

    nc = _get_nc()
    ones = np.ones((P, P), dtype=np.float32)
    in_maps = [
        {"x": x[c * BPC : (c + 1) * BPC], "w": w, "b": b, "ones": ones}
        for c in range(N_CORES)
    ]
    res = run_bass_kernel_spmd(nc, in_maps, list(range(N_CORES)))
    return np.concatenate([res.results[c]["out"] for c in range(N_CORES)], axis=0)


# revision 27
# speedup vs baseline: 11.2633x; 1.1365x over previous
"""Trainium2 Bass kernel for nn_AttentionLayer (B=64, F=1024, K=1024).

Reference computation (per batch b):
    scores[k, g] = sum_f input[b, f, k] * weight[f, g] + bias[g]
    alpha        = softmax(scores, axis=g)
    out[b, f, k] = input[b, f, k] * alpha[k, f]

Strategy: data-parallel over batch across 8 NeuronCores (8 batches/core).
Per batch, everything is computed in the transposed [g, k] layout so that no
transposes are ever needed:
    scoresT[g, k] = sum_f W[f, g] * X[f, k]      (lhsT = W chunk, rhs = X chunk)
    E[g, k]  = exp(scoresT + bias[g])            (ScalarE, bias is per-partition)
    S[., k]  = sum_g E[g, k]                     (matmul with ones[128,128]
                                                  stationary -> sum replicated
                                                  across partitions = free bcast)
    D = 1/S  (reciprocal_approx_fast)
    out[f, k] = X[f, k] * E[f, k] * D[k]         (VectorE, g === f axis)

Matmuls run with float32r operands (1 cyc/row vs 4 for fp32).
"""

import sys
from contextlib import ExitStack

import numpy as np

for _p in ("/opt/trn_rl_repo", "/root/.axon_site/_ro/trn_rl_repo"):
    if _p not in sys.path:
        sys.path.append(_p)

import concourse.bacc as bacc
import concourse.bass as bass
import concourse.mybir as mybir
import concourse.tile as tile
from concourse.bass_utils import run_bass_kernel_spmd

N_CORES = 8
B, F, K = 64, 1024, 1024
BPC = B // N_CORES            # batches per core
P = 128                       # SBUF partitions
NF = F // P                   # f (contraction) chunks
NG = F // P                   # g (feature/output-partition) chunks
KC = 512                      # moving free-dim chunk (fp32 max = 512)
NK = K // KC

FP32 = mybir.dt.float32
F32R = mybir.dt.float32r

EXP = mybir.ActivationFunctionType.Exp


def _build(mm_f32r: bool = True, bpc: int = BPC, reps: int = 1):
    nc = bacc.Bacc("TRN2", target_bir_lowering=False, debug=False)

    mmdt = F32R if mm_f32r else FP32
    x_d = nc.dram_tensor("x", [bpc, F, K], mmdt, kind="ExternalInput").ap()
    w_d = nc.dram_tensor("w", [F, F], mmdt, kind="ExternalInput").ap()
    b_d = nc.dram_tensor("b", [1, F], FP32, kind="ExternalInput").ap()
    ones_d = nc.dram_tensor("ones", [P, P], mmdt, kind="ExternalInput").ap()
    o_d = nc.dram_tensor("out", [bpc, F, K], FP32, kind="ExternalOutput").ap()

    def as_f32(ap):
        return ap.bitcast(FP32) if mm_f32r else ap

    with tile.TileContext(nc) as tc, ExitStack() as ctx:
        w_pool = ctx.enter_context(tc.tile_pool(name="w", bufs=1))
        c_pool = ctx.enter_context(tc.tile_pool(name="const", bufs=1))
        x_pool = ctx.enter_context(tc.tile_pool(name="x", bufs=16))
        e_pool = ctx.enter_context(tc.tile_pool(name="e", bufs=24))
        t_pool = ctx.enter_context(tc.tile_pool(name="t", bufs=3))
        d_pool = ctx.enter_context(tc.tile_pool(name="d", bufs=3))
        o_pool = ctx.enter_context(tc.tile_pool(name="o", bufs=4))
        sc_psum = ctx.enter_context(tc.tile_pool(name="sc", bufs=6, space="PSUM"))
        s_psum = ctx.enter_context(tc.tile_pool(name="s", bufs=2, space="PSUM"))

        # ---- constants (loaded once; W is interleaved with batch-0 X below)
        bias_sb = c_pool.tile([P, NG], FP32)
        nc.sync.dma_start(
            out=bias_sb[:], in_=b_d.rearrange("o (c p) -> (o p) c", p=P)
        )
        ones_sb = c_pool.tile([P, P], mmdt)
        nc.sync.dma_start(out=ones_sb[:], in_=ones_d)

        # w_sb[p, fc*F + g] = W[fc*128 + p, g]
        w_sb = w_pool.tile([P, NF * F], mmdt)

        def w_tile(fc, gc):
            off = fc * F + gc * P
            return w_sb[:, off : off + P]

        def prefetch_x(b, with_w=False):
            x_tiles = []
            for fc in range(NF):
                x_t = x_pool.tile([P, K], mmdt, tag="x")
                if with_w:
                    # startup: interleave W chunks with batch-0 X, and bring
                    # the kc=0 halves in first so slab 0 starts sooner
                    nc.sync.dma_start(
                        out=w_sb[:, fc * F : (fc + 1) * F],
                        in_=w_d[fc * P : (fc + 1) * P, :],
                    )
                    nc.sync.dma_start(
                        out=x_t[:, 0:KC], in_=x_d[b, fc * P : (fc + 1) * P, 0:KC]
                    )
                else:
                    nc.sync.dma_start(
                        out=x_t[:], in_=x_d[b, fc * P : (fc + 1) * P, :]
                    )
                x_tiles.append(x_t)
            if with_w:
                for fc in range(NF):
                    nc.sync.dma_start(
                        out=x_tiles[fc][:, KC:K],
                        in_=x_d[b, fc * P : (fc + 1) * P, KC:K],
                    )
            return x_tiles

        def slab_main(b, kc, x_tiles):
            """Main matmuls + exp + partial E-sum for one (batch, k-half) slab.

            The 8 exp tiles are accumulated with 7 DVE adds (running in the
            shadow of the matmuls) so the PE only does ONE ones-matmul per
            slab for the partition sum instead of 8.
            """
            ks = slice(kc * KC, (kc + 1) * KC)
            e_tiles = []
            t_t = None
            for gc in range(NG):
                sc = sc_psum.tile([P, KC], FP32, tag="sc")
                for fc in range(NF):
                    nc.tensor.matmul(
                        sc[:],
                        lhsT=w_tile(fc, gc),
                        rhs=x_tiles[fc][:, ks],
                        start=(fc == 0),
                        stop=(fc == NF - 1),
                    )
                e_t = e_pool.tile([P, KC], mmdt, tag="e")
                nc.scalar.activation(
                    e_t[:], sc[:], EXP, bias=bias_sb[:, gc : gc + 1], scale=1.0
                )
                e_tiles.append(e_t)
                if gc == 1:
                    t_t = t_pool.tile([P, KC], mmdt, tag="t")
                    nc.vector.tensor_add(
                        t_t[:], as_f32(e_tiles[0][:]), as_f32(e_t[:])
                    )
                elif gc > 1:
                    nc.vector.tensor_add(t_t[:], as_f32(t_t[:]), as_f32(e_t[:]))
            return e_tiles, t_t

        def slab_out(b, kc, x_tiles, e_tiles, t_t):
            """Partition-sum matmul + reciprocal + final muls + DMA out."""
            ks = slice(kc * KC, (kc + 1) * KC)
            s_t = s_psum.tile([P, KC], FP32, tag="s")
            nc.tensor.matmul(
                s_t[:], lhsT=ones_sb[:], rhs=t_t[:], start=True, stop=True
            )
            d_t = d_pool.tile([P, KC], FP32, tag="d")
            nc.vector.reciprocal_approx_fast(d_t[:], s_t[:])
            for fc in range(NF):
                o_t = o_pool.tile([P, KC], FP32, tag="o")
                nc.vector.tensor_mul(
                    o_t[:], as_f32(x_tiles[fc][:, ks]), as_f32(e_tiles[fc][:])
                )
                nc.vector.tensor_mul(o_t[:], o_t[:], d_t[:])
                nc.sync.dma_start(
                    out=o_d[b, fc * P : (fc + 1) * P, ks], in_=o_t[:]
                )

        # software pipeline over half-batch slabs: the PE stream is
        # [sums s-1][mains s][sums s][mains s+1]... so each slab's DVE chain
        # (recip + muls) overlaps the next slab's matmuls, and the kernel
        # tail after the very last main matmul is only one slab's epilogue.
        prev = None
        first = True
        for _ in range(reps):
            for b in range(bpc):
                x_tiles = prefetch_x(b, with_w=first)
                first = False
                for kc in range(NK):
                    if prev is not None:
                        slab_out(*prev)
                    e_tiles, t_t = slab_main(b, kc, x_tiles)
                    prev = (b, kc, x_tiles, e_tiles, t_t)
        slab_out(*prev)

    nc.compile()
    return nc


_NC = None


def _get_nc():
    global _NC
    if _NC is None:
        _NC = _build()
    return _NC


def kernel(**inputs) -> np.ndarray:
    x = np.ascontiguousarray(np.asarray(inputs["input"], dtype=np.float32))
    w = np.ascontiguousarray(np.asarray(inputs["weight"], dtype=np.float32))
    b = np.ascontiguousarray(np.asarray(inputs["bias"], dtype=np.float32))

    nc = _get_nc()
    ones = np.ones((P, P), dtype=np.float32)
    in_maps = [
        {"x": x[c * BPC : (c + 1) * BPC], "w": w, "b": b, "ones": ones}
        for c in range(N_CORES)
    ]
    res = run_bass_kernel_spmd(nc, in_maps, list(range(N_CORES)))
    return np.concatenate([res.results[c]["out"] for c in range(N_CORES)], axis=0)


# revision 28
# speedup vs baseline: 11.2737x; 1.0009x over previous
"""Trainium2 Bass kernel for nn_AttentionLayer (B=64, F=1024, K=1024).

Reference computation (per batch b):
    scores[k, g] = sum_f input[b, f, k] * weight[f, g] + bias[g]
    alpha        = softmax(scores, axis=g)
    out[b, f, k] = input[b, f, k] * alpha[k, f]

Strategy: data-parallel over batch across 8 NeuronCores (8 batches/core).
Per batch, everything is computed in the transposed [g, k] layout so that no
transposes are ever needed:
    scoresT[g, k] = sum_f W[f, g] * X[f, k]      (lhsT = W chunk, rhs = X chunk)
    E[g, k]  = exp(scoresT + bias[g])            (ScalarE, bias is per-partition)
    T[g, k]  = sum over the 8 g-chunk tiles      (7 VectorE adds, hidden under
                                                  the matmuls)
    S[., k]  = sum_g T[g, k]                     (ONE matmul with ones[128,128]
                                                  stationary -> sum replicated
                                                  across partitions = free bcast)
    D = 1/S  (reciprocal_approx_fast)
    out[f, k] = X[f, k] * E[f, k] * D[k]         (VectorE, g === f axis)

Matmuls run with float32r operands (1 cyc/row vs 4 for fp32; max rel err vs
the fp32 reference ~3.3e-4). The work is software-pipelined over half-batch
"slabs" (k split in two) so PE (~224us), DVE (~229us) and DMA (~216us) run
balanced at >85% occupancy; measured ~258us/core for the 8-batch shard.
"""

import sys
from contextlib import ExitStack

import numpy as np

for _p in ("/opt/trn_rl_repo", "/root/.axon_site/_ro/trn_rl_repo"):
    if _p not in sys.path:
        sys.path.append(_p)

import concourse.bacc as bacc
import concourse.bass as bass
import concourse.mybir as mybir
import concourse.tile as tile
from concourse.bass_utils import run_bass_kernel_spmd

N_CORES = 8
B, F, K = 64, 1024, 1024
BPC = B // N_CORES            # batches per core
P = 128                       # SBUF partitions
NF = F // P                   # f (contraction) chunks
NG = F // P                   # g (feature/output-partition) chunks
KC = 512                      # moving free-dim chunk (fp32 max = 512)
NK = K // KC

FP32 = mybir.dt.float32
F32R = mybir.dt.float32r

EXP = mybir.ActivationFunctionType.Exp


def _build(mm_f32r: bool = True, bpc: int = BPC, reps: int = 1):
    nc = bacc.Bacc("TRN2", target_bir_lowering=False, debug=False)

    mmdt = F32R if mm_f32r else FP32
    x_d = nc.dram_tensor("x", [bpc, F, K], mmdt, kind="ExternalInput").ap()
    w_d = nc.dram_tensor("w", [F, F], mmdt, kind="ExternalInput").ap()
    b_d = nc.dram_tensor("b", [1, F], FP32, kind="ExternalInput").ap()
    ones_d = nc.dram_tensor("ones", [P, P], mmdt, kind="ExternalInput").ap()
    o_d = nc.dram_tensor("out", [bpc, F, K], FP32, kind="ExternalOutput").ap()

    def as_f32(ap):
        return ap.bitcast(FP32) if mm_f32r else ap

    with tile.TileContext(nc) as tc, ExitStack() as ctx:
        w_pool = ctx.enter_context(tc.tile_pool(name="w", bufs=1))
        c_pool = ctx.enter_context(tc.tile_pool(name="const", bufs=1))
        x_pool = ctx.enter_context(tc.tile_pool(name="x", bufs=16))
        e_pool = ctx.enter_context(tc.tile_pool(name="e", bufs=24))
        t_pool = ctx.enter_context(tc.tile_pool(name="t", bufs=3))
        d_pool = ctx.enter_context(tc.tile_pool(name="d", bufs=3))
        o_pool = ctx.enter_context(tc.tile_pool(name="o", bufs=4))
        sc_psum = ctx.enter_context(tc.tile_pool(name="sc", bufs=6, space="PSUM"))
        s_psum = ctx.enter_context(tc.tile_pool(name="s", bufs=2, space="PSUM"))

        # ---- constants (loaded once; W is interleaved with batch-0 X below)
        bias_sb = c_pool.tile([P, NG], FP32)
        nc.sync.dma_start(
            out=bias_sb[:], in_=b_d.rearrange("o (c p) -> (o p) c", p=P)
        )
        ones_sb = c_pool.tile([P, P], mmdt)
        nc.sync.dma_start(out=ones_sb[:], in_=ones_d)

        # w_sb[p, fc*F + g] = W[fc*128 + p, g]
        w_sb = w_pool.tile([P, NF * F], mmdt)

        def w_tile(fc, gc):
            off = fc * F + gc * P
            return w_sb[:, off : off + P]

        def prefetch_x(b, with_w=False):
            x_tiles = []
            for fc in range(NF):
                x_t = x_pool.tile([P, K], mmdt, tag="x")
                if with_w:
                    # startup: interleave W chunks with batch-0 X, and bring
                    # the kc=0 halves in first so slab 0 starts sooner
                    nc.sync.dma_start(
                        out=w_sb[:, fc * F : (fc + 1) * F],
                        in_=w_d[fc * P : (fc + 1) * P, :],
                    )
                    nc.sync.dma_start(
                        out=x_t[:, 0:KC], in_=x_d[b, fc * P : (fc + 1) * P, 0:KC]
                    )
                else:
                    nc.sync.dma_start(
                        out=x_t[:], in_=x_d[b, fc * P : (fc + 1) * P, :]
                    )
                x_tiles.append(x_t)
            if with_w:
                for fc in range(NF):
                    nc.sync.dma_start(
                        out=x_tiles[fc][:, KC:K],
                        in_=x_d[b, fc * P : (fc + 1) * P, KC:K],
                    )
            return x_tiles

        def slab_main(b, kc, x_tiles):
            """Main matmuls + exp + partial E-sum for one (batch, k-half) slab.

            The 8 exp tiles are accumulated with 7 DVE adds (running in the
            shadow of the matmuls) so the PE only does ONE ones-matmul per
            slab for the partition sum instead of 8.
            """
            ks = slice(kc * KC, (kc + 1) * KC)
            e_tiles = []
            t_t = None
            for gc in range(NG):
                sc = sc_psum.tile([P, KC], FP32, tag="sc")
                for fc in range(NF):
                    nc.tensor.matmul(
                        sc[:],
                        lhsT=w_tile(fc, gc),
                        rhs=x_tiles[fc][:, ks],
                        start=(fc == 0),
                        stop=(fc == NF - 1),
                    )
                e_t = e_pool.tile([P, KC], mmdt, tag="e")
                nc.scalar.activation(
                    e_t[:], sc[:], EXP, bias=bias_sb[:, gc : gc + 1], scale=1.0
                )
                e_tiles.append(e_t)
                if gc == 1:
                    t_t = t_pool.tile([P, KC], mmdt, tag="t")
                    nc.vector.tensor_add(
                        t_t[:], as_f32(e_tiles[0][:]), as_f32(e_t[:])
                    )
                elif gc > 1:
                    nc.vector.tensor_add(t_t[:], as_f32(t_t[:]), as_f32(e_t[:]))
            return e_tiles, t_t

        def slab_out(b, kc, x_tiles, e_tiles, t_t):
            """Partition-sum matmul + reciprocal + final muls + DMA out."""
            ks = slice(kc * KC, (kc + 1) * KC)
            s_t = s_psum.tile([P, KC], FP32, tag="s")
            nc.tensor.matmul(
                s_t[:], lhsT=ones_sb[:], rhs=t_t[:], start=True, stop=True
            )
            d_t = d_pool.tile([P, KC], FP32, tag="d")
            nc.vector.reciprocal_approx_fast(d_t[:], s_t[:])
            for fc in range(NF):
                o_t = o_pool.tile([P, KC], FP32, tag="o")
                nc.vector.tensor_mul(
                    o_t[:], as_f32(x_tiles[fc][:, ks]), as_f32(e_tiles[fc][:])
                )
                nc.vector.tensor_mul(o_t[:], o_t[:], d_t[:])
                nc.sync.dma_start(
                    out=o_d[b, fc * P : (fc + 1) * P, ks], in_=o_t[:]
                )

        # software pipeline over half-batch slabs: the PE stream is
        # [sums s-1][mains s][sums s][mains s+1]... so each slab's DVE chain
        # (recip + muls) overlaps the next slab's matmuls, and the kernel
        # tail after the very last main matmul is only one slab's epilogue.
        prev = None
        first = True
        for _ in range(reps):
            for b in range(bpc):
                x_tiles = prefetch_x(b, with_w=first)
                first = False
                for kc in range(NK):
                    if prev is not None:
                        slab_out(*prev)
                    e_tiles, t_t = slab_main(b, kc, x_tiles)
                    prev = (b, kc, x_tiles, e_tiles, t_t)
        slab_out(*prev)

    nc.compile()
    return nc


_NC = None


def _get_nc():
    global _NC
    if _NC is None:
        _NC = _build()
    return _NC


def kernel(**inputs) -> np.ndarray:
    x = np.ascontiguousarray(np.asarray(inputs["input"], dtype=np.float32))
    w = np.ascontiguousarray(np.asarray(inputs["weight"], dtype=np.float32))
    b = np.ascontiguousarray(np.asarray(inputs["bias"], dtype=np.float32))

    nc = _get_nc()
    ones = np.ones((P, P), dtype=np.float32)
    in_maps = [
        {"x": x[c * BPC : (c + 1) * BPC], "w": w, "b": b, "ones": ones}
        for c in range(N_CORES)
    ]
    res = run_bass_kernel_spmd(nc, in_maps, list(range(N_CORES)))
    return np.concatenate([res.results[c]["out"] for c in range(N_CORES)], axis=0)
